# revision 30
# baseline (speedup 1.0000x reference)
"""Trainium2 Bass kernel: 3x3 stride-1 pad-1 Conv2d, 16->16 channels, 1024x1024.

Strategy (8 NeuronCores, spatial split over H):
  - Core i computes output rows [128*i, 128*i+128). 22 groups of 6 output rows
    (last group 2). Group t's rhs is one SBUF window of [128 partitions =
    (row r 0..7) x (channel c 0..15), 1032 free] holding the 8 input rows the
    6 outputs need (pre-padded columns so horizontal taps are free-dim shifts).
    The 3x3 conv is 3 accumulating matmuls (one per horizontal tap kw) per
    512-wide half against a block-banded [128,128] weight matrix:
    lhsT[(r,c),(g,o)] = W[o,c,kh=r-g,kw] for 0<=r-g<=2, g<6.
  - Input is staged on the host PARTITION-MAJOR and window-unrolled:
    xs[(r,c), (t, w)] fp16, so loads are a few block-DMAs with long
    contiguous descriptors. Early blocks are single groups (the ~2us DMA
    completion receipt dominates readiness at the pipeline start); later
    blocks are 4 groups.
  - PSUM [128,512] fp32 per half -> cast-copy to fp16 SBUF staging, halves
    split across Vector and Scalar engines -> coalesced DMA (up to 4 groups
    per transfer, shrinking toward the end so the tail drains early) to a
    partition-major fp16 DRAM output (host reassembles).
  - Dummy matmuls on a scratch tile run while the first input block loads,
    flipping the PE HAM clock gate to 2.4 GHz before real work starts.
"""

import sys

sys.path.insert(0, "/opt/trn_rl_repo")

# Under axon, bass_utils' trace path hard-imports antenv.axon_hooks, which
# some images lack (boot degrades silently, bass_utils then crashes). Shim
# it in and best-effort register the real ctypes NTFF hook so trace=True
# works (or at worst degrades gracefully) in any container.
try:
    import antenv.axon_hooks  # noqa: F401
except ImportError:
    import types as _types

    _m = _types.ModuleType("antenv.axon_hooks")
    _m._hook = None
    _m.set_axon_ntff_profile_hook = lambda h, _m=_m: setattr(_m, "_hook", h)
    _m.get_axon_ntff_profile_hook = lambda _m=_m: _m._hook
    sys.modules["antenv.axon_hooks"] = _m
    try:
        from trn_agent_boot.trn_boot import _ntff_profile_via_ctypes

        _m._hook = _ntff_profile_via_ctypes("/opt/axon/libaxon_pjrt.so")
    except Exception:
        pass
except Exception:
    pass

import numpy as np

import concourse.bass as bass  # noqa: F401  (engine handles live on nc)
import concourse.mybir as mybir
import concourse.tile as tile
from concourse import bacc
from concourse.bass_utils import run_bass_kernel_spmd

C = 16          # channels in/out
H = 1024        # image height/width
W = 1024
NCORES = 8
RPC = H // NCORES       # output rows per core = 128
ADV = 6                 # output rows per group
GROUPS = (RPC + ADV - 1) // ADV   # 22 groups (last partial: 2 rows)
WPAD = 1032             # padded row width (col 0 is zero, 1..1024 data)
IN_BLOCKS = (1, 1, 1, 1, 2, 4, 4, 4, 4)  # groups per input block-DMA
OUT_BLOCKS = (4, 4, 4, 4, 2, 2, 1, 1)  # groups per output block-DMA

_CACHE = {}


def _build_nc(warm: int = 6):
    key = ("nc", warm)
    if key in _CACHE:
        return _CACHE[key]
    nc = bacc.Bacc("TRN2", target_bir_lowering=False, debug=False)
    f32 = mybir.dt.float32
    f16 = mybir.dt.float16
    # xs columns: [0:384) packed weights, then the 22 window-unrolled groups.
    # Packing the weights into the head of block 0's first DMA makes the
    # first real matmul gate on a single transfer.
    xs = nc.dram_tensor(
        "xs", [128, 3 * 128 + GROUPS * WPAD], f16, kind="ExternalInput"
    ).ap()
    out = nc.dram_tensor("out", [96, GROUPS * W], f16, kind="ExternalOutput").ap()
    WOFF = 3 * 128  # column offset of window 0 within xs

    with tile.TileContext(nc) as tc:
        with (
            tc.tile_pool(name="scr", bufs=1) as scr,
            tc.tile_pool(name="xin", bufs=len(IN_BLOCKS)) as xin,
            tc.tile_pool(name="ps", bufs=3, space="PSUM") as ps,
            tc.tile_pool(name="wps", bufs=1, space="PSUM") as wpsp,
            tc.tile_pool(name="ost", bufs=4) as ostp,
        ):
            # --- input block loads (partition-major, window-unrolled) ---
            # Block 0's first DMA carries the packed weights plus window 0's
            # h0 columns, so one transfer gates the first real matmul; its
            # second DMA brings the rest of window 0. Each early block is a
            # single group because the ~2us DMA completion receipt dominates
            # readiness at the pipeline start.
            xtiles = []   # (tile, first_group)
            t0 = 0
            for nb in IN_BLOCKS:
                if t0 == 0:
                    xt = xin.tile([128, WOFF + WPAD], f16)
                    nc.sync.dma_start(
                        out=xt[:, 0 : WOFF + 516], in_=xs[:, 0 : WOFF + 516]
                    )
                    nc.sync.dma_start(
                        out=xt[:, WOFF + 516 : WOFF + WPAD],
                        in_=xs[:, WOFF + 516 : WOFF + WPAD],
                    )
                    wt = xt[:, 0:WOFF]
                else:
                    xt = xin.tile([128, nb * WPAD], f16)
                    nc.sync.dma_start(
                        out=xt,
                        in_=xs[:, WOFF + t0 * WPAD : WOFF + (t0 + nb) * WPAD],
                    )
                xtiles.append((xt, t0))
                t0 += nb

            # --- PE warmup: flip HAM to 2.4 GHz while input block 0 loads ---
            if warm:
                ws = scr.tile([128, 512], f16)
                nc.vector.memset(ws, 0.0)
                wpsum = wpsp.tile([128, 512], f32)
                for _ in range(warm):
                    nc.tensor.matmul(
                        wpsum, ws[:, 0:128], ws, start=True, stop=True
                    )

            # --- main loop ---
            blk_of_group = []
            for bi, nb in enumerate(IN_BLOCKS):
                blk_of_group += [bi] * nb

            t0 = 0
            for ob, nb in enumerate(OUT_BLOCKS):
                final = ob == len(OUT_BLOCKS) - 1 and nb == 1
                ost = ostp.tile([96, nb * W], f16)
                for j in range(nb):
                    t = t0 + j
                    xt, tstart = xtiles[blk_of_group[t]]
                    base = (t - tstart) * WPAD + (WOFF if blk_of_group[t] == 0 else 0)
                    # one 2-bank PSUM tile per group (each matmul still
                    # targets a single bank via the half slices) — halves
                    # the PSUM alloc/semaphore traffic on the PE queue
                    pt2 = ps.tile([128, 1024], f32)
                    for h in range(2):
                        pt = pt2[:, h * 512 : (h + 1) * 512]
                        for kw in range(3):
                            nc.tensor.matmul(
                                pt,
                                wt[:, kw * 128 : (kw + 1) * 128],
                                xt[:, base + h * 512 + kw : base + h * 512 + kw + 512],
                                start=(kw == 0),
                                stop=(kw == 2),
                            )
                        if not final:
                            dst = ost[0:96, j * W + h * 512 : j * W + (h + 1) * 512]
                            # the block right before the final group swaps
                            # engines so ACT's queue is empty when the final
                            # group's tail quarter lands on it
                            on_dve = (h == 0) ^ (ob == len(OUT_BLOCKS) - 2)
                            if on_dve:
                                nc.vector.tensor_copy(dst, pt[0:96])
                            else:
                                nc.scalar.activation(
                                    dst, pt[0:96],
                                    mybir.ActivationFunctionType.Copy,
                                )
                        elif h == 0:
                            nc.vector.tensor_copy(ost[0:96, 0:512], pt[0:96])
                        else:
                            # final group's h1 in two quarters on two engines
                            # so the tail-exposed copy and DMA are minimal
                            nc.vector.tensor_copy(
                                ost[0:96, 512:768], pt[0:96, 0:256]
                            )
                            nc.scalar.activation(
                                ost[0:96, 768:1024], pt[0:96, 256:512],
                                mybir.ActivationFunctionType.Copy,
                            )
                # out-DMAs on the SP ring (idle after the input blocks
                # issue); the final group drains in three pieces across two
                # rings so the tail waits only on a 256-col quarter
                if final:
                    nc.sync.dma_start(
                        out=out[:, t0 * W : t0 * W + 512], in_=ost[:, 0:512]
                    )
                    nc.sync.dma_start(
                        out=out[:, t0 * W + 512 : t0 * W + 768],
                        in_=ost[:, 512:768],
                    )
                    nc.scalar.dma_start(
                        out=out[:, t0 * W + 768 : (t0 + 1) * W],
                        in_=ost[:, 768:W],
                    )
                else:
                    nc.sync.dma_start(
                        out=out[:, t0 * W : (t0 + nb) * W],
                        in_=ost[:, 0 : nb * W],
                    )
                t0 += nb
    nc.compile()
    _CACHE[key] = nc
    return nc


def _pack_weights(weight: np.ndarray) -> np.ndarray:
    """wpk[(r*16+c), kw*128 + (g*16+o)] = W[o,c,r-g,kw] for 0<=r-g<=2, g<6."""
    wpk = np.zeros((8, C, 3, 8, C), dtype=np.float32)  # [r, c, kw, g, o]
    wt = weight.astype(np.float32).transpose(1, 3, 0, 2)  # [c, kw, o, kh]
    for g in range(ADV):
        for kh in range(3):
            wpk[g + kh, :, :, g, :] = wt[:, :, :, kh]
    return np.ascontiguousarray(wpk.reshape(128, 3 * 128))


def _slice_inputs(x: np.ndarray) -> list[np.ndarray]:
    """Per-core window-unrolled partition-major slices [128, GROUPS*WPAD] fp16.

    xs[(r*16+c), t*WPAD + w] = xpad[128*i + 6*t + r, c, w] where xpad has one
    zero row on top and one zero col on the left (plus right-edge zeros).
    """
    xr = x[0].transpose(1, 0, 2)  # [H, C, W]
    gpad = np.zeros((H + 8, C, WPAD), dtype=np.float32)
    gpad[1 : H + 1, :, 1 : W + 1] = xr
    rows = 6 * np.arange(GROUPS)[None, :] + np.arange(8)[:, None]  # [8, 22]
    res = []
    for i in range(NCORES):
        win = gpad[RPC * i + rows]             # [8, 22, C, WPAD]
        win = win.transpose(0, 2, 1, 3)        # [8, C, 22, WPAD]
        res.append(np.ascontiguousarray(
            win.reshape(128, GROUPS * WPAD).astype(np.float16)))
    return res


def kernel(x: np.ndarray, weight: np.ndarray, _run_kw: dict | None = None):
    nc = _build_nc()
    wpk = _pack_weights(weight).astype(np.float16)
    slices = _slice_inputs(np.asarray(x, dtype=np.float32))
    in_maps = [
        {"xs": np.ascontiguousarray(np.concatenate([wpk, s], axis=1))}
        for s in slices
    ]
    res = run_bass_kernel_spmd(
        nc, in_maps, core_ids=list(range(NCORES)), **(_run_kw or {})
    )
    full = np.empty((C, H, W), dtype=np.float32)
    for i in range(NCORES):
        o = np.asarray(res.results[i]["out"], dtype=np.float32)  # [96, 22*W]
        o = o.reshape(96, GROUPS, W).transpose(1, 0, 2)          # [22, 96, W]
        o = o.reshape(GROUPS, ADV, C, W).transpose(2, 0, 1, 3)   # [C, 22, 6, W]
        full[:, RPC * i : RPC * (i + 1), :] = o.reshape(C, GROUPS * ADV, W)[:, :RPC]
    if _run_kw:
        kernel.last_results = res
    return full


# revision 31
# speedup vs baseline: 1.0862x; 1.0862x over previous
"""Trainium2 Bass kernel: 3x3 stride-1 pad-1 Conv2d, 16->16 channels, 1024x1024.

Strategy (8 NeuronCores, spatial split over H):
  - Core i computes output rows [128*i, 128*i+128). 22 groups of 6 output rows
    (last group 2). Group t's rhs is one SBUF window of [128 partitions =
    (row r 0..7) x (channel c 0..15), 1032 free] holding the 8 input rows the
    6 outputs need (pre-padded columns so horizontal taps are free-dim shifts).
    The 3x3 conv is 3 accumulating matmuls (one per horizontal tap kw) per
    512-wide half against a block-banded [128,128] weight matrix:
    lhsT[(r,c),(g,o)] = W[o,c,kh=r-g,kw] for 0<=r-g<=2, g<6.
  - Input is staged on the host PARTITION-MAJOR and window-unrolled:
    xs[(r,c), (t, w)] fp16, so loads are a few block-DMAs with long
    contiguous descriptors. Early blocks are single groups (the ~2us DMA
    completion receipt dominates readiness at the pipeline start); later
    blocks are 4 groups.
  - PSUM [128,512] fp32 per half -> cast-copy to fp16 SBUF staging, halves
    split across Vector and Scalar engines -> coalesced DMA (up to 4 groups
    per transfer, shrinking toward the end so the tail drains early) to a
    partition-major fp16 DRAM output (host reassembles).
  - Dummy matmuls on a scratch tile run while the first input block loads,
    flipping the PE HAM clock gate to 2.4 GHz before real work starts.
"""

import sys

sys.path.insert(0, "/opt/trn_rl_repo")

# Under axon, bass_utils' trace path hard-imports antenv.axon_hooks, which
# some images lack (boot degrades silently, bass_utils then crashes). Shim
# it in and best-effort register the real ctypes NTFF hook so trace=True
# works (or at worst degrades gracefully) in any container.
try:
    import antenv.axon_hooks  # noqa: F401
except ImportError:
    import types as _types

    _m = _types.ModuleType("antenv.axon_hooks")
    _m._hook = None
    _m.set_axon_ntff_profile_hook = lambda h, _m=_m: setattr(_m, "_hook", h)
    _m.get_axon_ntff_profile_hook = lambda _m=_m: _m._hook
    sys.modules["antenv.axon_hooks"] = _m
    try:
        from trn_agent_boot.trn_boot import _ntff_profile_via_ctypes

        _m._hook = _ntff_profile_via_ctypes("/opt/axon/libaxon_pjrt.so")
    except Exception:
        pass
except Exception:
    pass

import numpy as np

import concourse.bass as bass  # noqa: F401  (engine handles live on nc)
import concourse.mybir as mybir
import concourse.tile as tile
from concourse import bacc
from concourse.bass_utils import run_bass_kernel_spmd

C = 16          # channels in/out
H = 1024        # image height/width
W = 1024
NCORES = 8
RPC = H // NCORES       # output rows per core = 128
ADV = 6                 # output rows per group
GROUPS = (RPC + ADV - 1) // ADV   # 22 groups (last partial: 2 rows)
WPAD = 1032             # padded row width (col 0 is zero, 1..1024 data)
IN_BLOCKS = (1, 1, 1, 1, 2, 4, 4, 4, 4)  # groups per input block-DMA
OUT_BLOCKS = (4, 4, 4, 4, 2, 2, 1, 1)  # groups per output block-DMA

_CACHE = {}


def _build_nc(warm: int = 6):
    key = ("nc", warm)
    if key in _CACHE:
        return _CACHE[key]
    nc = bacc.Bacc("TRN2", target_bir_lowering=False, debug=False)
    f32 = mybir.dt.float32
    f16 = mybir.dt.float16
    # xs columns: [0:384) packed weights, then the 22 window-unrolled groups.
    # Packing the weights into the head of block 0's first DMA makes the
    # first real matmul gate on a single transfer.
    xs = nc.dram_tensor(
        "xs", [128, 3 * 128 + GROUPS * WPAD], f16, kind="ExternalInput"
    ).ap()
    out = nc.dram_tensor("out", [96, GROUPS * W], f16, kind="ExternalOutput").ap()
    WOFF = 3 * 128  # column offset of window 0 within xs

    with tile.TileContext(nc) as tc:
        with (
            tc.tile_pool(name="scr", bufs=1) as scr,
            tc.tile_pool(name="xin", bufs=len(IN_BLOCKS)) as xin,
            tc.tile_pool(name="ps", bufs=6, space="PSUM") as ps,
            tc.tile_pool(name="wps", bufs=1, space="PSUM") as wpsp,
            tc.tile_pool(name="ost", bufs=4) as ostp,
        ):
            # --- input block loads (partition-major, window-unrolled) ---
            # Block 0's first DMA carries the packed weights plus window 0's
            # h0 columns, so one transfer gates the first real matmul; its
            # second DMA brings the rest of window 0. Each early block is a
            # single group because the ~2us DMA completion receipt dominates
            # readiness at the pipeline start.
            xtiles = []   # (tile, first_group)
            t0 = 0
            for nb in IN_BLOCKS:
                if t0 == 0:
                    xt = xin.tile([128, WOFF + WPAD], f16)
                    nc.sync.dma_start(
                        out=xt[:, 0 : WOFF + 516], in_=xs[:, 0 : WOFF + 516]
                    )
                    nc.sync.dma_start(
                        out=xt[:, WOFF + 516 : WOFF + WPAD],
                        in_=xs[:, WOFF + 516 : WOFF + WPAD],
                    )
                    wt = xt[:, 0:WOFF]
                else:
                    xt = xin.tile([128, nb * WPAD], f16)
                    nc.sync.dma_start(
                        out=xt,
                        in_=xs[:, WOFF + t0 * WPAD : WOFF + (t0 + nb) * WPAD],
                    )
                xtiles.append((xt, t0))
                t0 += nb

            # --- PE warmup: flip HAM to 2.4 GHz while input block 0 loads ---
            if warm:
                ws = scr.tile([128, 512], f16)
                nc.vector.memset(ws, 0.0)
                wpsum = wpsp.tile([128, 512], f32)
                for _ in range(warm):
                    nc.tensor.matmul(
                        wpsum, ws[:, 0:128], ws, start=True, stop=True
                    )

            # --- main loop ---
            blk_of_group = []
            for bi, nb in enumerate(IN_BLOCKS):
                blk_of_group += [bi] * nb

            t0 = 0
            for ob, nb in enumerate(OUT_BLOCKS):
                final = ob == len(OUT_BLOCKS) - 1 and nb == 1
                ost = ostp.tile([96, nb * W], f16)
                for j in range(nb):
                    t = t0 + j
                    xt, tstart = xtiles[blk_of_group[t]]
                    base = (t - tstart) * WPAD + (WOFF if blk_of_group[t] == 0 else 0)
                    for h in range(2):
                        pt = ps.tile([128, 512], f32)
                        for kw in range(3):
                            nc.tensor.matmul(
                                pt,
                                wt[:, kw * 128 : (kw + 1) * 128],
                                xt[:, base + h * 512 + kw : base + h * 512 + kw + 512],
                                start=(kw == 0),
                                stop=(kw == 2),
                            )
                        if not final:
                            dst = ost[0:96, j * W + h * 512 : j * W + (h + 1) * 512]
                            # the block right before the final group swaps
                            # engines so ACT's queue is empty when the final
                            # group's tail quarter lands on it
                            on_dve = (h == 0) ^ (ob == len(OUT_BLOCKS) - 2)
                            if on_dve:
                                nc.vector.tensor_copy(dst, pt[0:96])
                            else:
                                nc.scalar.activation(
                                    dst, pt[0:96],
                                    mybir.ActivationFunctionType.Copy,
                                )
                        elif h == 0:
                            nc.vector.tensor_copy(ost[0:96, 0:512], pt[0:96])
                        else:
                            # final group's h1 in two quarters on two engines
                            # so the tail-exposed copy and DMA are minimal
                            nc.vector.tensor_copy(
                                ost[0:96, 512:768], pt[0:96, 0:256]
                            )
                            nc.scalar.activation(
                                ost[0:96, 768:1024], pt[0:96, 256:512],
                                mybir.ActivationFunctionType.Copy,
                            )
                # out-DMAs on the SP ring (idle after the input blocks
                # issue); the final group drains in three pieces across two
                # rings so the tail waits only on a 256-col quarter
                if final:
                    nc.sync.dma_start(
                        out=out[:, t0 * W : t0 * W + 512], in_=ost[:, 0:512]
                    )
                    nc.sync.dma_start(
                        out=out[:, t0 * W + 512 : t0 * W + 768],
                        in_=ost[:, 512:768],
                    )
                    nc.scalar.dma_start(
                        out=out[:, t0 * W + 768 : (t0 + 1) * W],
                        in_=ost[:, 768:W],
                    )
                else:
                    nc.sync.dma_start(
                        out=out[:, t0 * W : (t0 + nb) * W],
                        in_=ost[:, 0 : nb * W],
                    )
                t0 += nb
    nc.compile()
    _CACHE[key] = nc
    return nc


def _pack_weights(weight: np.ndarray) -> np.ndarray:
    """wpk[(r*16+c), kw*128 + (g*16+o)] = W[o,c,r-g,kw] for 0<=r-g<=2, g<6."""
    wpk = np.zeros((8, C, 3, 8, C), dtype=np.float32)  # [r, c, kw, g, o]
    wt = weight.astype(np.float32).transpose(1, 3, 0, 2)  # [c, kw, o, kh]
    for g in range(ADV):
        for kh in range(3):
            wpk[g + kh, :, :, g, :] = wt[:, :, :, kh]
    return np.ascontiguousarray(wpk.reshape(128, 3 * 128))


def _slice_inputs(x: np.ndarray) -> list[np.ndarray]:
    """Per-core window-unrolled partition-major slices [128, GROUPS*WPAD] fp16.

    xs[(r*16+c), t*WPAD + w] = xpad[128*i + 6*t + r, c, w] where xpad has one
    zero row on top and one zero col on the left (plus right-edge zeros).
    """
    xr = x[0].transpose(1, 0, 2)  # [H, C, W]
    gpad = np.zeros((H + 8, C, WPAD), dtype=np.float32)
    gpad[1 : H + 1, :, 1 : W + 1] = xr
    rows = 6 * np.arange(GROUPS)[None, :] + np.arange(8)[:, None]  # [8, 22]
    res = []
    for i in range(NCORES):
        win = gpad[RPC * i + rows]             # [8, 22, C, WPAD]
        win = win.transpose(0, 2, 1, 3)        # [8, C, 22, WPAD]
        res.append(np.ascontiguousarray(
            win.reshape(128, GROUPS * WPAD).astype(np.float16)))
    return res


def kernel(x: np.ndarray, weight: np.ndarray, _run_kw: dict | None = None):
    nc = _build_nc()
    wpk = _pack_weights(weight).astype(np.float16)
    slices = _slice_inputs(np.asarray(x, dtype=np.float32))
    in_maps = [
        {"xs": np.ascontiguousarray(np.concatenate([wpk, s], axis=1))}
        for s in slices
    ]
    res = run_bass_kernel_spmd(
        nc, in_maps, core_ids=list(range(NCORES)), **(_run_kw or {})
    )
    full = np.empty((C, H, W), dtype=np.float32)
    for i in range(NCORES):
        o = np.asarray(res.results[i]["out"], dtype=np.float32)  # [96, 22*W]
        o = o.reshape(96, GROUPS, W).transpose(1, 0, 2)          # [22, 96, W]
        o = o.reshape(GROUPS, ADV, C, W).transpose(2, 0, 1, 3)   # [C, 22, 6, W]
        full[:, RPC * i : RPC * (i + 1), :] = o.reshape(C, GROUPS * ADV, W)[:, :RPC]
    if _run_kw:
        kernel.last_results = res
    return full
